# revision 15
# baseline (speedup 1.0000x reference)
"""Trainium2 Bass kernel for nn_ClassBasedSMDecoder.

Reference computation (N=8192 tokens, H=1024 hid, C=1024 classes, K=32):
    p_class = x @ W_cls.T + b_cls                      # [N, C]
    p_words = einsum('nh,nkh->nk', x, W_words[cls]) + b_words[cls]   # [N, K]

Sharding over 8 NeuronCores:
  * p_class: data-parallel over tokens — core i computes tokens
    [i*1024, (i+1)*1024) against the full (replicated) W_cls.
  * p_words: expert-parallel — core i owns classes [i*128, (i+1)*128).
    The host routes tokens to their class's core. Classes are sorted by
    token count (descending) per core, grouped 4 per PE pass with a
    per-group capacity (max count over the group across all cores), and
    4 groups per "block" (the DMA/scheduling unit). Each PE pass does
    one full-width stationary load (4 classes x 32 words = 128 columns)
    and streams the group's 4*cap routed-token columns, computing a
    [128, 4*cap] PSUM block whose 4 diagonal [32, cap] sub-blocks are
    the wanted logits (host discards the off-diagonal waste).

All matmul inputs are cast to bf16 on the host (fp32 accumulate in PSUM);
p_class is returned from the device in bf16.

DRAM layouts are partition-major: [128, ...] with the contraction chunk
index folded into the free dimension, so every tensor (or class-block)
loads with a single large contiguous DMA.
"""

import numpy as np
import ml_dtypes

import concourse.bass as bass
import concourse.mybir as mybir
import concourse.tile as tile
from concourse import bacc
from concourse.bass_utils import run_bass_kernel_spmd

BF16 = ml_dtypes.bfloat16

N, H, C, K = 8192, 1024, 1024, 32
NCORES = 8
CS = C // NCORES        # 128 classes per core
TOK = N // NCORES       # 1024 tokens per core (p_class shard)
HC = H // 128           # 8 contraction chunks
NGRP = CS // 4          # 32 groups of 4 classes
NBLK = 8                # 4 groups per block
GPB = NGRP // NBLK      # groups per block

_cache: dict = {}


def _build(caps: tuple):
    """Build + compile the per-core Bass program for group capacities `caps`."""
    caps = list(caps)
    assert len(caps) == NGRP
    gws = [4 * c for c in caps]              # group widths (tokens)
    capsum = sum(caps)
    gw_off = np.concatenate([[0], np.cumsum(gws)])   # within full token space
    bws = [sum(gws[b * GPB:(b + 1) * GPB]) for b in range(NBLK)]

    dt = mybir.dt
    nc = bacc.Bacc(
        "TRN2", target_bir_lowering=False, debug=False, enable_asserts=False
    )

    # xt/wct: [128, half, HC, 512] — each half loads as one contiguous DMA
    xt = nc.dram_tensor("xt", [128, HC * TOK], dt.bfloat16, kind="ExternalInput")
    wct = nc.dram_tensor("wct", [128, HC * C], dt.bfloat16, kind="ExternalInput")
    wwt = nc.dram_tensor(
        "wwt", [128, NBLK * HC * GPB * 128], dt.bfloat16, kind="ExternalInput"
    )
    xgt = nc.dram_tensor("xgt", [128, HC * 4 * capsum], dt.bfloat16,
                         kind="ExternalInput")
    pc = nc.dram_tensor("pc", [TOK, C], dt.bfloat16, kind="ExternalOutput")
    pw = nc.dram_tensor("pw", [128, 4 * capsum], dt.bfloat16, kind="ExternalOutput")

    # sanity: SBUF per-partition budget (bytes)
    sbuf_bytes = (HC * TOK + HC * C + NBLK * HC * GPB * 128 + HC * 4 * capsum) * 2 \
        + 4 * capsum * 4 + 4 * 512 * 2
    assert sbuf_bytes < 190 * 1024, f"SBUF budget exceeded: {sbuf_bytes}"

    with tile.TileContext(nc) as tc:
        with (
            tc.tile_pool(name="big", bufs=1) as big,
            tc.tile_pool(name="stage", bufs=4) as stage,
            tc.tile_pool(name="ps_pc", bufs=4, space=bass.MemorySpace.PSUM) as ps_pc,
            tc.tile_pool(name="ps_pw", bufs=4, space=bass.MemorySpace.PSUM) as ps_pw,
        ):
            HF = HC * 512
            xt_s = [big.tile([128, HF], dt.bfloat16, name=f"xt_s{m}",
                             tag=f"xt_s{m}") for m in range(2)]
            wct_s = [big.tile([128, HF], dt.bfloat16, name=f"wct_s{m}",
                              tag=f"wct_s{m}") for m in range(2)]
            wwt_s = [
                big.tile([128, HC * GPB * 128], dt.bfloat16, name=f"wwt_s{b}",
                         tag=f"wwt_s{b}")
                for b in range(NBLK)
            ]
            xgt_s = [
                big.tile([128, HC * bws[b]], dt.bfloat16, name=f"xgt_s{b}",
                         tag=f"xgt_s{b}")
                for b in range(NBLK)
            ]
            pw_split = gw_off[NGRP // 2]
            pwst = [
                big.tile([128, int(pw_split), ], dt.bfloat16, name="pwst_a"),
                big.tile([128, 4 * capsum - int(pw_split)], dt.bfloat16,
                         name="pwst_b"),
            ]

            # Loads, in consumption order, each one large contiguous DMA.
            nc.sync.dma_start(wct_s[0][:], wct[:, 0:HF])
            nc.sync.dma_start(xt_s[0][:], xt[:, 0:HF])
            nc.sync.dma_start(wct_s[1][:], wct[:, HF:2 * HF])
            nc.sync.dma_start(xt_s[1][:], xt[:, HF:2 * HF])
            xgt_doff = [0]
            for b in range(NBLK):
                nc.sync.dma_start(
                    wwt_s[b][:],
                    wwt[:, b * HC * GPB * 128:(b + 1) * HC * GPB * 128],
                )
                nc.sync.dma_start(
                    xgt_s[b][:], xgt[:, xgt_doff[-1]:xgt_doff[-1] + HC * bws[b]]
                )
                xgt_doff.append(xgt_doff[-1] + HC * bws[b])

            def xt_slice(mt, h):
                return xt_s[mt // 4][:, h * 512 + (mt % 4) * 128:
                                     h * 512 + (mt % 4) * 128 + 128]

            def pc_store(mt, ct, acc):
                st = stage.tile([128, 512], dt.bfloat16, tag="pcst")
                nc.vector.tensor_copy(st[:], acc[:])
                nc.scalar.dma_start(
                    pc[mt * 128:(mt + 1) * 128, ct * 512:(ct + 1) * 512], st[:]
                )

            def pc_single(mt, ct):
                acc = ps_pc.tile([128, 512], dt.float32, tag="pcacc")
                for h in range(HC):
                    nc.tensor.matmul(
                        acc[:],
                        xt_slice(mt, h),
                        wct_s[ct][:, h * 512:(h + 1) * 512],
                        start=(h == 0),
                        stop=(h == HC - 1),
                    )
                pc_store(mt, ct, acc)

            def pc_pair(mt):
                # both ct halves share each h-chunk's stationary operand
                acc = [ps_pc.tile([128, 512], dt.float32, name=f"pcacc{ct}",
                                  tag="pcacc") for ct in range(2)]
                for h in range(HC):
                    for ct in range(2):
                        nc.tensor.matmul(
                            acc[ct][:],
                            xt_slice(mt, h),
                            wct_s[ct][:, h * 512:(h + 1) * 512],
                            start=(h == 0),
                            stop=(h == HC - 1),
                        )
                for ct in range(2):
                    pc_store(mt, ct, acc[ct])

            def pw_block(b):
                for gl in range(GPB):
                    g = b * GPB + gl
                    gw = gws[g]
                    goff = gw_off[g] - gw_off[b * GPB]   # within block
                    acc = ps_pw.tile([128, gw], dt.float32, tag="pwacc")
                    for h in range(HC):
                        nc.tensor.matmul(
                            acc[:],
                            wwt_s[b][:, h * GPB * 128 + gl * 128:
                                      h * GPB * 128 + (gl + 1) * 128],
                            xgt_s[b][:, h * bws[b] + goff:h * bws[b] + goff + gw],
                            start=(h == 0),
                            stop=(h == HC - 1),
                        )
                    half, hoff = (0, 0) if g < NGRP // 2 else (1, pw_split)
                    nc.vector.tensor_copy(
                        pwst[half][:, gw_off[g] - hoff:gw_off[g] - hoff + gw],
                        acc[:],
                    )

            # Schedule: early singles while half-loads land, pairs after,
            # p_words blocks interleaved as their data arrives.
            sched = [("s", (0, 0)), ("s", (1, 0)), ("p", 2), ("p", 3),
                     ("pw", 0), ("p", 4), ("pw", 1), ("p", 5),
                     ("pw", 2), ("p", 6), ("pw", 3), ("p", 7),
                     ("pw", 4), ("s", (0, 1)), ("pw", 5), ("s", (1, 1)),
                     ("pw", 6), ("pw", 7)]
            for kind, arg in sched:
                if kind == "s":
                    pc_single(*arg)
                elif kind == "p":
                    pc_pair(arg)
                else:
                    pw_block(arg)
                    if arg == NBLK // 2 - 1:
                        nc.scalar.dma_start(pw[:, 0:int(pw_split)], pwst[0][:])
            nc.scalar.dma_start(pw[:, int(pw_split):], pwst[1][:])

    nc.compile()
    return nc


def _part_major(a, f):
    """[H, F] -> [128, HC*F] with chunk h at cols [h*F, (h+1)*F)."""
    return np.ascontiguousarray(
        a.reshape(HC, 128, f).transpose(1, 0, 2).reshape(128, HC * f)
    )


def _plan(cls):
    """Routing plan: per-core class sort, group capacities, token selection."""
    counts = np.bincount(cls, minlength=C).reshape(NCORES, CS)
    perm = np.argsort(-counts, axis=1, kind="stable")        # rank -> local class
    sc = np.take_along_axis(counts, perm, 1)                 # sorted counts desc
    caps = sc[:, ::4].max(axis=0)                            # [NGRP]
    caps = np.maximum(caps + (caps & 1), 2).astype(np.int64)  # even, >= 2
    # token lists per class
    order = np.argsort(cls, kind="stable")
    flat_counts = counts.reshape(-1)
    starts = np.zeros(C, np.int64)
    np.cumsum(flat_counts[:-1], out=starts[1:])
    return counts, perm, caps, order, starts


def _prepare(x, cls, W_cls, W_words):
    cls = np.asarray(cls).astype(np.int64)
    xb = np.asarray(x).astype(BF16)
    xbT = np.ascontiguousarray(xb.T)                         # [H, N]
    counts, perm, caps, order, starts = _plan(cls)
    capsum = int(caps.sum())

    wcT_full = W_cls.astype(BF16).T                          # [H, C]
    wcT = np.concatenate(
        [_part_major(np.ascontiguousarray(wcT_full[:, m * 512:(m + 1) * 512]), 512)
         for m in range(2)], axis=1)

    wwb = W_words.astype(BF16)                               # [C, K, H]

    in_maps = []
    sels = []
    for i in range(NCORES):
        xt_i = np.concatenate(
            [_part_major(
                np.ascontiguousarray(
                    xbT[:, i * TOK + m * 512:i * TOK + (m + 1) * 512]), 512)
             for m in range(2)], axis=1)
        # routed tokens, sorted-class order, padded per group capacity
        sel_i = np.zeros((CS, int(caps.max())), np.int64)
        valid_i = np.zeros((CS, int(caps.max())), bool)
        tok_cols = np.zeros(4 * capsum, np.int64)
        colpos = 0
        for r in range(CS):
            c_local = perm[i, r]
            c_glob = i * CS + c_local
            cnt = counts[i, c_local]
            cap = int(caps[r // 4])
            toks = order[starts[c_glob]:starts[c_glob] + cnt]
            sel_i[r, :cnt] = toks
            valid_i[r, :cnt] = True
            tok_cols[colpos:colpos + cnt] = toks
            colpos += cap
        sels.append((sel_i, valid_i))

        # xgt: per block, [H, bw] -> [128, HC*bw], concat blocks
        gws = 4 * caps
        bw_split = np.concatenate([[0], np.cumsum(gws.reshape(NBLK, GPB).sum(1))])
        parts = []
        for b in range(NBLK):
            lo, hi = int(bw_split[b]), int(bw_split[b + 1])
            parts.append(_part_major(
                np.ascontiguousarray(xbT[:, tok_cols[lo:hi]]), hi - lo
            ))
        xgt_i = np.concatenate(parts, axis=1)

        # wwt: per block, classes in sorted order
        parts = []
        for b in range(NBLK):
            ranks = perm[i, b * GPB * 4:(b + 1) * GPB * 4]
            Wb = wwb[i * CS + ranks]                          # [16, K, H]
            arr = np.ascontiguousarray(
                Wb.transpose(2, 0, 1).reshape(H, GPB * 4 * K)
            )
            parts.append(_part_major(arr, GPB * 4 * K))
        wwt_i = np.concatenate(parts, axis=1)

        in_maps.append({"xt": xt_i, "wct": wcT, "wwt": wwt_i, "xgt": xgt_i})
    return in_maps, (caps, perm, sels)


def _assemble(results, meta, cls, b_cls, b_words):
    caps, perm, sels = meta
    p_class = np.concatenate(
        [results[i]["pc"].astype(np.float32) for i in range(NCORES)], axis=0
    )
    p_class = p_class + np.asarray(b_cls)[None, :].astype(np.float32)

    gw_off = np.concatenate([[0], np.cumsum(4 * caps)])
    p_words = np.empty((N, K), np.float32)
    for i in range(NCORES):
        pw_i = results[i]["pw"].astype(np.float32)            # [128, 4*capsum]
        sel_i, valid_i = sels[i]
        for r in range(CS):
            g, j = r // 4, r % 4
            cap = int(caps[g])
            nv = int(valid_i[r].sum())
            if nv == 0:
                continue
            base = int(gw_off[g]) + j * cap
            blk = pw_i[32 * j:32 * (j + 1), base:base + nv]    # [K, nv]
            p_words[sel_i[r, :nv]] = blk.T
    p_words = p_words + np.asarray(b_words).astype(np.float32)[np.asarray(cls)]
    return p_class, p_words


def run(inputs, trace=False, trace_kwargs=None):
    """Run the SPMD kernel on 8 cores. Returns ((p_class, p_words), results)."""
    x = np.asarray(inputs["x"])
    cls = np.asarray(inputs["cls"])
    in_maps, meta = _prepare(x, cls, inputs["W_cls"], inputs["W_words"])
    key = tuple(int(c) for c in meta[0])
    if key not in _cache:
        _cache[key] = _build(key)
    nc = _cache[key]
    res = run_bass_kernel_spmd(
        nc,
        in_maps,
        list(range(NCORES)),
        trace=trace,
        **(trace_kwargs or {}),
    )
    out = _assemble(res.results, meta, cls, inputs["b_cls"], inputs["b_words"])
    return out, res


def kernel(**inputs):
    (p_class, p_words), _ = run(inputs)
    return p_class, p_words


# revision 18
# speedup vs baseline: 1.0734x; 1.0734x over previous
"""Trainium2 Bass kernel for nn_ClassBasedSMDecoder.

Reference computation (N=8192 tokens, H=1024 hid, C=1024 classes, K=32):
    p_class = x @ W_cls.T + b_cls                      # [N, C]
    p_words = einsum('nh,nkh->nk', x, W_words[cls]) + b_words[cls]   # [N, K]

Sharding over 8 NeuronCores:
  * p_class: data-parallel over tokens — core i computes tokens
    [i*1024, (i+1)*1024) against the full (replicated) W_cls.
  * p_words: expert-parallel — core i owns classes [i*128, (i+1)*128).
    The host routes tokens to their class's core. Classes are sorted by
    token count (descending) per core, grouped 4 per PE pass with a
    per-group capacity (max count over the group across all cores), and
    4 groups per "block" (the DMA/scheduling unit). Each PE pass does
    one full-width stationary load (4 classes x 32 words = 128 columns)
    and streams the group's 4*cap routed-token columns, computing a
    [128, 4*cap] PSUM block whose 4 diagonal [32, cap] sub-blocks are
    the wanted logits (host discards the off-diagonal waste).

All matmul inputs are cast to bf16 on the host (fp32 accumulate in PSUM);
p_class is returned from the device in bf16.

DRAM layouts are partition-major: [128, ...] with the contraction chunk
index folded into the free dimension, so every tensor (or class-block)
loads with a single large contiguous DMA.
"""

import numpy as np
import ml_dtypes

import concourse.bass as bass
import concourse.mybir as mybir
import concourse.tile as tile
from concourse import bacc
from concourse.bass_utils import run_bass_kernel_spmd

BF16 = ml_dtypes.bfloat16

N, H, C, K = 8192, 1024, 1024, 32
NCORES = 8
CS = C // NCORES        # 128 classes per core
TOK = N // NCORES       # 1024 tokens per core (p_class shard)
HC = H // 128           # 8 contraction chunks
NGRP = CS // 4          # 32 groups of 4 classes
NBLK = 8                # 4 groups per block
GPB = NGRP // NBLK      # groups per block

_cache: dict = {}


def _build(caps: tuple):
    """Build + compile the per-core Bass program for group capacities `caps`."""
    caps = list(caps)
    assert len(caps) == NGRP
    gws = [4 * c for c in caps]              # group widths (tokens)
    capsum = sum(caps)
    gw_off = np.concatenate([[0], np.cumsum(gws)])   # within full token space
    bws = [sum(gws[b * GPB:(b + 1) * GPB]) for b in range(NBLK)]

    dt = mybir.dt
    nc = bacc.Bacc(
        "TRN2", target_bir_lowering=False, debug=False, enable_asserts=False
    )

    # xt/wct: [128, half, HC, 512] — each half loads as one contiguous DMA
    xt = nc.dram_tensor("xt", [128, HC * TOK], dt.bfloat16, kind="ExternalInput")
    wct = nc.dram_tensor("wct", [128, HC * C], dt.bfloat16, kind="ExternalInput")
    wwt = nc.dram_tensor(
        "wwt", [128, NBLK * HC * GPB * 128], dt.bfloat16, kind="ExternalInput"
    )
    xgt = nc.dram_tensor("xgt", [128, HC * 4 * capsum], dt.bfloat16,
                         kind="ExternalInput")
    pc = nc.dram_tensor("pc", [TOK, C], dt.bfloat16, kind="ExternalOutput")
    pw = nc.dram_tensor("pw", [128, 4 * capsum], dt.bfloat16, kind="ExternalOutput")

    # sanity: SBUF per-partition budget (bytes)
    sbuf_bytes = (HC * TOK + HC * C + NBLK * HC * GPB * 128 + HC * 4 * capsum) * 2 \
        + 4 * capsum * 4 + 4 * 512 * 2
    assert sbuf_bytes < 190 * 1024, f"SBUF budget exceeded: {sbuf_bytes}"

    with tile.TileContext(nc) as tc:
        with (
            tc.tile_pool(name="big", bufs=1) as big,
            tc.tile_pool(name="stage", bufs=4) as stage,
            tc.tile_pool(name="ps_pc", bufs=6, space=bass.MemorySpace.PSUM) as ps_pc,
            tc.tile_pool(name="ps_pw", bufs=2, space=bass.MemorySpace.PSUM) as ps_pw,
        ):
            HF = HC * 512
            xt_s = [big.tile([128, HF], dt.bfloat16, name=f"xt_s{m}",
                             tag=f"xt_s{m}") for m in range(2)]
            wct_s = big.tile([128, HC * C], dt.bfloat16, name="wct_s")
            wwt_s = [
                big.tile([128, HC * GPB * 128], dt.bfloat16, name=f"wwt_s{b}",
                         tag=f"wwt_s{b}")
                for b in range(NBLK)
            ]
            xgt_s = [
                big.tile([128, HC * bws[b]], dt.bfloat16, name=f"xgt_s{b}",
                         tag=f"xgt_s{b}")
                for b in range(NBLK)
            ]
            pw_split = int(gw_off[NGRP // 2])
            pwst = [
                big.tile([128, pw_split], dt.bfloat16, name="pwst_a"),
                big.tile([128, 4 * capsum - pw_split], dt.bfloat16,
                         name="pwst_b"),
            ]

            # Loads, in consumption order, each one large contiguous DMA.
            nc.sync.dma_start(xt_s[0][:], xt[:, 0:HF])
            nc.sync.dma_start(wct_s[:], wct[:])
            nc.sync.dma_start(xt_s[1][:], xt[:, HF:2 * HF])
            xgt_doff = [0]
            for b in range(NBLK):
                nc.sync.dma_start(
                    wwt_s[b][:],
                    wwt[:, b * HC * GPB * 128:(b + 1) * HC * GPB * 128],
                )
                nc.sync.dma_start(
                    xgt_s[b][:], xgt[:, xgt_doff[-1]:xgt_doff[-1] + HC * bws[b]]
                )
                xgt_doff.append(xgt_doff[-1] + HC * bws[b])

            def pc_tile(mt):
                # two N=512 matmuls per h-chunk sharing the stationary operand
                acc = [ps_pc.tile([128, 512], dt.float32, name=f"pcacc{ct}",
                                  tag="pcacc") for ct in range(2)]
                for h in range(HC):
                    for ct in range(2):
                        nc.tensor.matmul(
                            acc[ct][:],
                            xt_s[mt // 4][:, h * 512 + (mt % 4) * 128:
                                          h * 512 + (mt % 4) * 128 + 128],
                            wct_s[:, h * C + ct * 512:h * C + (ct + 1) * 512],
                            start=(h == 0),
                            stop=(h == HC - 1),
                        )
                st = stage.tile([128, 1024], dt.bfloat16, tag="pcst")
                for ct in range(2):
                    nc.vector.tensor_copy(
                        st[:, ct * 512:(ct + 1) * 512], acc[ct][:]
                    )
                nc.scalar.dma_start(pc[mt * 128:(mt + 1) * 128, :], st[:])

            def pw_block(b):
                for gl in range(GPB):
                    g = b * GPB + gl
                    gw = gws[g]
                    goff = gw_off[g] - gw_off[b * GPB]   # within block
                    acc = ps_pw.tile([128, gw], dt.float32, tag="pwacc")
                    for h in range(HC):
                        nc.tensor.matmul(
                            acc[:],
                            wwt_s[b][:, h * GPB * 128 + gl * 128:
                                      h * GPB * 128 + (gl + 1) * 128],
                            xgt_s[b][:, h * bws[b] + goff:h * bws[b] + goff + gw],
                            start=(h == 0),
                            stop=(h == HC - 1),
                        )
                    half, hoff = (0, 0) if g < NGRP // 2 else (1, pw_split)
                    nc.vector.tensor_copy(
                        pwst[half][:, gw_off[g] - hoff:gw_off[g] - hoff + gw],
                        acc[:],
                    )

            # Schedule: p_class tiles as soon as xt half 0 + wct land,
            # p_words blocks interleaved as their data arrives, small tail.
            sched = [("pc", 0), ("pc", 1), ("pc", 2), ("pc", 3),
                     ("pw", 0), ("pc", 4), ("pw", 1), ("pc", 5),
                     ("pw", 2), ("pc", 6), ("pw", 3), ("pc", 7),
                     ("pw", 4), ("pw", 5), ("pw", 6), ("pw", 7)]
            for kind, arg in sched:
                if kind == "pc":
                    pc_tile(arg)
                else:
                    pw_block(arg)
                    if arg == NBLK // 2 - 1:
                        nc.scalar.dma_start(pw[:, 0:pw_split], pwst[0][:])
            nc.scalar.dma_start(pw[:, pw_split:], pwst[1][:])

    nc.compile()
    return nc


def _part_major(a, f):
    """[H, F] -> [128, HC*F] with chunk h at cols [h*F, (h+1)*F)."""
    return np.ascontiguousarray(
        a.reshape(HC, 128, f).transpose(1, 0, 2).reshape(128, HC * f)
    )


def _plan(cls):
    """Routing plan: per-core class sort, group capacities, token selection."""
    counts = np.bincount(cls, minlength=C).reshape(NCORES, CS)
    perm = np.argsort(-counts, axis=1, kind="stable")        # rank -> local class
    sc = np.take_along_axis(counts, perm, 1)                 # sorted counts desc
    caps = sc[:, ::4].max(axis=0)                            # [NGRP]
    caps = np.maximum(caps + (caps & 1), 2).astype(np.int64)  # even, >= 2
    # token lists per class
    order = np.argsort(cls, kind="stable")
    flat_counts = counts.reshape(-1)
    starts = np.zeros(C, np.int64)
    np.cumsum(flat_counts[:-1], out=starts[1:])
    return counts, perm, caps, order, starts


def _prepare(x, cls, W_cls, W_words):
    cls = np.asarray(cls).astype(np.int64)
    xb = np.asarray(x).astype(BF16)
    xbT = np.ascontiguousarray(xb.T)                         # [H, N]
    counts, perm, caps, order, starts = _plan(cls)
    capsum = int(caps.sum())

    wcT = _part_major(np.ascontiguousarray(W_cls.astype(BF16).T), C)

    wwb = W_words.astype(BF16)                               # [C, K, H]

    in_maps = []
    sels = []
    for i in range(NCORES):
        xt_i = np.concatenate(
            [_part_major(
                np.ascontiguousarray(
                    xbT[:, i * TOK + m * 512:i * TOK + (m + 1) * 512]), 512)
             for m in range(2)], axis=1)
        # routed tokens, sorted-class order, padded per group capacity
        sel_i = np.zeros((CS, int(caps.max())), np.int64)
        valid_i = np.zeros((CS, int(caps.max())), bool)
        tok_cols = np.zeros(4 * capsum, np.int64)
        colpos = 0
        for r in range(CS):
            c_local = perm[i, r]
            c_glob = i * CS + c_local
            cnt = counts[i, c_local]
            cap = int(caps[r // 4])
            toks = order[starts[c_glob]:starts[c_glob] + cnt]
            sel_i[r, :cnt] = toks
            valid_i[r, :cnt] = True
            tok_cols[colpos:colpos + cnt] = toks
            colpos += cap
        sels.append((sel_i, valid_i))

        # xgt: per block, [H, bw] -> [128, HC*bw], concat blocks
        gws = 4 * caps
        bw_split = np.concatenate([[0], np.cumsum(gws.reshape(NBLK, GPB).sum(1))])
        parts = []
        for b in range(NBLK):
            lo, hi = int(bw_split[b]), int(bw_split[b + 1])
            parts.append(_part_major(
                np.ascontiguousarray(xbT[:, tok_cols[lo:hi]]), hi - lo
            ))
        xgt_i = np.concatenate(parts, axis=1)

        # wwt: per block, classes in sorted order
        parts = []
        for b in range(NBLK):
            ranks = perm[i, b * GPB * 4:(b + 1) * GPB * 4]
            Wb = wwb[i * CS + ranks]                          # [16, K, H]
            arr = np.ascontiguousarray(
                Wb.transpose(2, 0, 1).reshape(H, GPB * 4 * K)
            )
            parts.append(_part_major(arr, GPB * 4 * K))
        wwt_i = np.concatenate(parts, axis=1)

        in_maps.append({"xt": xt_i, "wct": wcT, "wwt": wwt_i, "xgt": xgt_i})
    return in_maps, (caps, perm, sels)


def _assemble(results, meta, cls, b_cls, b_words):
    caps, perm, sels = meta
    p_class = np.concatenate(
        [results[i]["pc"].astype(np.float32) for i in range(NCORES)], axis=0
    )
    p_class = p_class + np.asarray(b_cls)[None, :].astype(np.float32)

    gw_off = np.concatenate([[0], np.cumsum(4 * caps)])
    p_words = np.empty((N, K), np.float32)
    for i in range(NCORES):
        pw_i = results[i]["pw"].astype(np.float32)            # [128, 4*capsum]
        sel_i, valid_i = sels[i]
        for r in range(CS):
            g, j = r // 4, r % 4
            cap = int(caps[g])
            nv = int(valid_i[r].sum())
            if nv == 0:
                continue
            base = int(gw_off[g]) + j * cap
            blk = pw_i[32 * j:32 * (j + 1), base:base + nv]    # [K, nv]
            p_words[sel_i[r, :nv]] = blk.T
    p_words = p_words + np.asarray(b_words).astype(np.float32)[np.asarray(cls)]
    return p_class, p_words


def run(inputs, trace=False, trace_kwargs=None):
    """Run the SPMD kernel on 8 cores. Returns ((p_class, p_words), results)."""
    x = np.asarray(inputs["x"])
    cls = np.asarray(inputs["cls"])
    in_maps, meta = _prepare(x, cls, inputs["W_cls"], inputs["W_words"])
    key = tuple(int(c) for c in meta[0])
    if key not in _cache:
        _cache[key] = _build(key)
    nc = _cache[key]
    res = run_bass_kernel_spmd(
        nc,
        in_maps,
        list(range(NCORES)),
        trace=trace,
        **(trace_kwargs or {}),
    )
    out = _assemble(res.results, meta, cls, inputs["b_cls"], inputs["b_words"])
    return out, res


def kernel(**inputs):
    (p_class, p_words), _ = run(inputs)
    return p_class, p_words
